# revision 18
# baseline (speedup 1.0000x reference)
"""Trainium2 Bass kernel: gradient of the EnergyAttention scalar energy.

reference:
    q = einsum('bqd,hzd->bqhz', g, wq); k = einsum('bkd,hzd->bkhz', g, wk)
    scores = einsum('bqhz,bkhz->bhqk', q, k)
    E = -(logsumexp(BETA*scores, -1)/BETA).sum() + POS_SCALE*(g*pos).sum()
    out = dE/dg

Math: with P = softmax(BETA*scores) per (b,h,q):
    out[b] = -sum_h [ (P@K) @ wq_h + (P.T@Qn) @ wk_h ] + POS_SCALE*pos
where Qn = diag(1/Z) Q (row-normalized by the softmax partition Z).

Sharding: 8 cores; core c handles batch b=c//4 and heads 4*(c%4)..4*(c%4)+3
(two head-pairs packed into the 128-partition dim).  Each core emits the
positive partial 16*sum_h[(dQ)wq + (dK)wk] of shape [S, D] in fp16; the
host combines (unscale, negate, sum over head-groups, add positional term).

v6 structure. HW facts (measured via NTFF microbenchmarks):
  - fp16 512-row matmul ~290ns clean, ~400ns next to ACT activity, and
    620-760ns when open PSUM accumulation groups coexist with ACT.
  - ACT exp of [128,1024] ~1.1us on the single scalar engine.
  - The DMA XBAR (dma_start_transpose) transposes fp16 [128,1024] in
    ~0.9us on the otherwise-idle DMA engines.
So P^T is produced by XBAR-transposing P (no scoresT matmuls, no second
exp pass -- halves the ACT floor), and Qraw/K2n come from XBAR too (no
PE transposes).  PT lives as [k128, (i, j, q128)] so every transpose
writes a contiguous block (non-contiguous XBAR dst is broken on HW).
  prelude(p): proj -> QT2/KT2 fp16; XBAR -> qraw_w/k2n_w [s128,(sb,z2)]
  loop(p) i:  scores(i) [4 clean MMs] -> exp_P(i) (Z accum)
              XBAR P(i,a) -> PT_all; dK(i-1) closed blocks + DVE adds
  pair end:   dQ burst (PSUM-accumulated over k-blocks; ACT idle)
  out:        gout = 16 * sum_h dQ wq + dK wk  (fp16)
"""

import numpy as np

B = 2
S = 1024
D = 1024
NH = 16
Z = 64
BETA = 1.0 / np.sqrt(np.float32(Z))
POS_SCALE = 0.001
N_CORES = 8
HPC = 4           # heads per core
NPAIR = 2         # head pairs per core
ND = D // 128     # 8 d-tiles
NQ = S // 128     # 8 q/k blocks
NCH = S // 512    # 2 moving-dim chunks
SC_G = 16.0       # dK/dQ prescale (gout comes out x SC_G)

_CACHE = {}


def build_nc(reps=1):
    """Build the (SPMD, identical-per-core) Bass program."""
    from contextlib import ExitStack

    import concourse.mybir as mybir
    import concourse.tile as tile
    from concourse import bacc
    F32 = mybir.dt.float32
    F16 = mybir.dt.float16
    MUL = mybir.AluOpType.mult
    ADD = mybir.AluOpType.add
    EXP = mybir.ActivationFunctionType.Exp

    nc = bacc.Bacc(
        "TRN2",
        target_bir_lowering=False,
        debug=False,
        enable_asserts=False,
        num_devices=N_CORES,
    )

    xT = nc.dram_tensor("xT", [D, S], F16, kind="ExternalInput").ap()
    wqT2 = nc.dram_tensor("wqT2", [NPAIR * D, 128], F16, kind="ExternalInput").ap()
    wkT2 = nc.dram_tensor("wkT2", [NPAIR * D, 128], F16, kind="ExternalInput").ap()
    wq2n = nc.dram_tensor("wq2n", [NPAIR * 128, D], F16, kind="ExternalInput").ap()
    wk2n = nc.dram_tensor("wk2n", [NPAIR * 128, D], F16, kind="ExternalInput").ap()
    gout = nc.dram_tensor("gout", [S, D], F16, kind="ExternalOutput").ap()

    with tile.TileContext(nc) as tc, ExitStack() as ctx:
        sb1 = ctx.enter_context(tc.tile_pool(name="sb1", bufs=1))
        sb2 = ctx.enter_context(tc.tile_pool(name="sb2", bufs=2))
        sb4 = ctx.enter_context(tc.tile_pool(name="sb4", bufs=4))
        pp = ctx.enter_context(tc.tile_pool(name="pp", bufs=8))
        # PSUM: "ps_sc" 2x[128,1024]f32 (4 banks): proj, scores ring,
        # outproj, z-transpose.  "ps_dkq" 2x[128,1024]f32 (4 banks):
        # dK(i) closed blocks (DVE-drained) and the pair-end dQ burst
        # accumulator.
        ps_sc = ctx.enter_context(tc.tile_pool(name="ps_sc", bufs=2, space="PSUM"))
        ps_dkq = ctx.enter_context(tc.tile_pool(name="ps_dkq", bufs=2, space="PSUM"))


        for _rep in range(reps):
            # ---- input loads (ordered so pair-0 proj can start ASAP) -----
            gt = sb1.tile([128, ND * S], F16, tag="gt")  # G^T: [d_in_tile, (dt, s)]
            wtq = sb1.tile([128, NPAIR * ND * 128], F16, tag="wtq")  # [d, (pair,dt,z2)]
            wtk = sb1.tile([128, NPAIR * ND * 128], F16, tag="wtk")
            nc.sync.dma_start(wtk[:, 0:128], wkT2[0:128, :].rearrange("(b p) c -> p (b c)", p=128))
            nc.sync.dma_start(gt[:, 0:S], xT[0:128, :])
            nc.sync.dma_start(
                wtk[:, 128 : ND * 128].rearrange("p (b c) -> p b c", b=ND - 1),
                wkT2[128:D, :].rearrange("(b p) c -> p b c", p=128),
            )
            for dt in range(1, ND):
                nc.sync.dma_start(
                    gt[:, dt * S : (dt + 1) * S], xT[dt * 128 : (dt + 1) * 128, :]
                )
                if dt == 1:
                    nc.sync.dma_start(
                        wtq[:, 0 : ND * 128].rearrange("p (b c) -> p b c", b=ND),
                        wqT2[0:D, :].rearrange("(b p) c -> p b c", p=128),
                    )

            nc.sync.dma_start(
                wtq[:, ND * 128 :].rearrange("p (b c) -> p b c", b=ND),
                wqT2[D:, :].rearrange("(b p) c -> p b c", p=128),
            )
            nc.sync.dma_start(
                wtk[:, ND * 128 :].rearrange("p (b c) -> p b c", b=ND),
                wkT2[D:, :].rearrange("(b p) c -> p b c", p=128),
            )
            wnq = sb1.tile([128, NPAIR * D], F16, tag="wnq")  # [z2, (pair, d)]
            wnk = sb1.tile([128, NPAIR * D], F16, tag="wnk")
            for p in range(NPAIR):
                nc.sync.dma_start(wnq[:, p * D : (p + 1) * D], wq2n[p * 128 : (p + 1) * 128, :])
                nc.sync.dma_start(wnk[:, p * D : (p + 1) * D], wk2n[p * 128 : (p + 1) * 128, :])

            # persistent across pairs
            dqt2 = sb1.tile([128, NPAIR * S], F16, tag="dqt2")  # [z2, (pair, q)]
            dkt2 = sb1.tile([128, NPAIR * S], F16, tag="dkt2")  # [z2, (pair, k)]

            for p in range(NPAIR):
                # ---- prelude: projections QT2/KT2 [z2, s] fp16 --------------
                qt2 = sb2.tile([128, S], F16, tag="qt2")
                kt2 = sb2.tile([128, S], F16, tag="kt2")
                for wt, dst in ((wtk, kt2), (wtq, qt2)):
                    ps = ps_sc.tile([128, S], F32, tag="ps_sc", name=f"pj{p}_{dst.tensor.name}")
                    for dt in range(ND):
                        j = p * ND + dt
                        for ch in range(NCH):
                            nc.tensor.matmul(
                                ps[:, ch * 512 : (ch + 1) * 512],
                                lhsT=wt[:, j * 128 : (j + 1) * 128],
                                rhs=gt[:, dt * S + ch * 512 : dt * S + ch * 512 + 512],
                                start=(dt == 0),
                                stop=(dt == ND - 1),
                            )
                    for ch in range(NCH):
                        nc.vector.tensor_copy(
                            dst[:, ch * 512 : (ch + 1) * 512],
                            ps[:, ch * 512 : (ch + 1) * 512],
                        )

                # ---- XBAR transposes: qraw_w/k2n_w [s128, (sb, z2)] ---------
                qraw = sb2.tile([128, S], F16, tag="qraw")
                k2n = sb2.tile([128, S], F16, tag="k2n")
                nc.sync.dma_start_transpose(
                    qraw[:].rearrange("p (sb z) -> p sb z", sb=NQ), qt2[:]
                )
                nc.sync.dma_start_transpose(
                    k2n[:].rearrange("p (sb z) -> p sb z", sb=NQ), kt2[:]
                )

                # ---- window: scores -> exp_P -> XBAR(P) + dK blocks ---------
                zsum2 = sb2.tile([128, 16], F32, tag="zsum2")  # [(q), (head, qb)]
                dkacc = sb2.tile([128, S], F32, tag="dkacc")
                # PT layout [k128, (a, i, j, q128)]: each XBAR writes the
                # contiguous [128, (j, q128)] block for its (a, i).
                PT_all = sb2.tile([128, 2 * NQ * S], F16, tag="PT_all", bufs=1)
                P_hist = {}

                def emit_scores(i):
                    tiles = []
                    for a in range(2):
                        ps = ps_sc.tile([128, S], F32, tag="ps_sc", name=f"sc{p}_{i}_{a}")
                        for ch in range(NCH):
                            nc.tensor.matmul(
                                ps[:, ch * 512 : (ch + 1) * 512],
                                lhsT=qt2[a * 64 : (a + 1) * 64, i * 128 : (i + 1) * 128],
                                rhs=kt2[a * 64 : (a + 1) * 64, ch * 512 : (ch + 1) * 512],
                                start=True,
                                stop=True,
                            )
                        tiles.append(ps)
                    return tiles

                def emit_exp_P(i, pt_s):
                    # Pn = (16/Z) * exp(beta*s): the q-row 1/Z attaches to P,
                    # so neither dK nor dQ needs any later normalization.
                    for a in range(2):
                        pb = pp.tile([128, S], F16, tag="P", name=f"P{p}_{i}_{a}")
                        nc.scalar.activation(
                            pb[:],
                            pt_s[a][:],
                            EXP,
                            scale=float(BETA),
                            accum_out=zsum2[:, a * NQ + i : a * NQ + i + 1],
                        )
                        zq = sb4.tile([128, 1], F32, tag="zq", name=f"zq{p}_{i}_{a}")
                        nc.vector.reciprocal(zq[:], zsum2[:, a * NQ + i : a * NQ + i + 1])
                        nc.vector.tensor_scalar(
                            pb[:], pb[:], zq[:], float(SC_G), MUL, MUL
                        )
                        P_hist[(i, a)] = pb
                        # PT block for (a, i): [k128, (j, q128)] contiguous
                        nc.sync.dma_start_transpose(
                            PT_all[:, (a * NQ + i) * S : (a * NQ + i + 1) * S]
                            .rearrange("p (j q) -> p j q", j=NQ),
                            pb[:],
                        )

                def emit_dk(i):
                    """dK block i: closed matmuls into PSUM, DVE-accumulated."""
                    ps = ps_dkq.tile([128, S], F32, tag="ps_dkq", name=f"dkb{p}_{i}")
                    for a in range(2):
                        Pb = P_hist.pop((i, a))
                        for ch in range(NCH):
                            nc.tensor.matmul(
                                ps[a * 64 : (a + 1) * 64, ch * 512 : (ch + 1) * 512],
                                lhsT=qraw[:, i * 128 + a * 64 : i * 128 + (a + 1) * 64],
                                rhs=Pb[:, ch * 512 : (ch + 1) * 512],
                                start=True,
                                stop=True,
                                skip_group_check=True,
                            )
                    if i == 0:
                        nc.vector.tensor_copy(dkacc[:], ps[:])
                    else:
                        nc.vector.tensor_tensor(dkacc[:], dkacc[:], ps[:], ADD)

                for i in range(NQ):
                    pt_s = emit_scores(i)
                    emit_exp_P(i, pt_s)
                    if i > 0:
                        emit_dk(i - 1)
                emit_dk(NQ - 1)

                nc.vector.tensor_copy(dkt2[:, p * S : (p + 1) * S], dkacc[:])

                # ---- pair-end dQ burst: dQ^T += K^T_j @ PT_j over k-blocks --
                # (PSUM accumulation; ACT is idle here so chains run clean)
                dq_ps = ps_dkq.tile([128, S], F32, tag="ps_dkq", name=f"dq_ps{p}")
                PTr = PT_all[:].rearrange("p (a i j q) -> p a i j q", a=2, i=NQ, j=NQ)
                for j in range(NQ):
                    for a in range(2):
                        for ch in range(NCH):
                            # moving: q-blocks i in [4ch, 4ch+4), each 128 wide
                            nc.tensor.matmul(
                                dq_ps[a * 64 : (a + 1) * 64, ch * 512 : (ch + 1) * 512],
                                lhsT=k2n[:, j * 128 + a * 64 : j * 128 + (a + 1) * 64],
                                rhs=PTr[:, a, 4 * ch : 4 * ch + 4, j, :],
                                start=(j == 0),
                                stop=(j == NQ - 1),
                                skip_group_check=True,
                            )
                nc.vector.tensor_copy(dqt2[:, p * S : (p + 1) * S], dq_ps[:])

            # ---- output projection  16 * sum_h dQ wq + dK wk ----------------
            for sb in range(NQ):
                ps = ps_sc.tile([128, S], F32, tag="ps_sc", name=f"op{sb}")
                for ch in range(NCH):
                    # accumulation order: latest-ready operand (dqt2 pair 1)
                    # last, so the chain can start as soon as pair 0 is done
                    n = 0
                    for pq in range(NPAIR):
                        for dmat, wmat in ((dkt2, wnk), (dqt2, wnq)):
                            nc.tensor.matmul(
                                ps[:, ch * 512 : (ch + 1) * 512],
                                lhsT=dmat[:, pq * S + sb * 128 : pq * S + (sb + 1) * 128],
                                rhs=wmat[:, pq * D + ch * 512 : pq * D + ch * 512 + 512],
                                start=(n == 0),
                                stop=(n == 2 * NPAIR - 1),
                            )
                            n += 1
                go = sb4.tile([128, S], F16, tag="go", name=f"go{sb}")
                nc.vector.tensor_copy(go[:], ps[:])
                nc.sync.dma_start(gout[sb * 128 : (sb + 1) * 128, :], go[:])

    nc.compile()
    return nc


def core_inputs(x, wq, wk, core):
    """Per-core input arrays (host-side shard/layout prep)."""
    b = core // 4
    h0 = 4 * (core % 4)
    xT = np.ascontiguousarray(x[b].T).astype(np.float16)
    wqT2 = np.empty((NPAIR * D, 128), np.float16)
    wkT2 = np.empty((NPAIR * D, 128), np.float16)
    wq2n = np.empty((NPAIR * 128, D), np.float16)
    wk2n = np.empty((NPAIR * 128, D), np.float16)
    for p in range(NPAIR):
        ha, hb = h0 + 2 * p, h0 + 2 * p + 1
        wqT2[p * D : (p + 1) * D, 0:64] = wq[ha].T
        wqT2[p * D : (p + 1) * D, 64:128] = wq[hb].T
        wkT2[p * D : (p + 1) * D, 0:64] = wk[ha].T
        wkT2[p * D : (p + 1) * D, 64:128] = wk[hb].T
        wq2n[p * 128 : p * 128 + 64] = wq[ha]
        wq2n[p * 128 + 64 : (p + 1) * 128] = wq[hb]
        wk2n[p * 128 : p * 128 + 64] = wk[ha]
        wk2n[p * 128 + 64 : (p + 1) * 128] = wk[hb]
    return {"xT": xT, "wqT2": wqT2, "wkT2": wkT2, "wq2n": wq2n, "wk2n": wk2n}


def combine(gouts):
    """Host unshard: unscale, negate, all-reduce over head groups, add pos."""
    pos = np.linspace(-0.5, 0.5, S, dtype=np.float32)[:, None] * np.float32(POS_SCALE)
    out = np.empty((B, S, D), np.float32)
    for b in range(B):
        acc = np.zeros((S, D), np.float64)
        for c in range(4 * b, 4 * b + 4):
            acc += np.asarray(gouts[c], np.float64)
        out[b] = (pos.astype(np.float64) - acc / SC_G).astype(np.float32)
    return out


def kernel(x, wq, wk, trace=False):
    x = np.asarray(x, np.float32)
    wq = np.asarray(wq, np.float32)
    wk = np.asarray(wk, np.float32)
    if "nc" not in _CACHE:
        _CACHE["nc"] = build_nc()
    nc = _CACHE["nc"]
    from concourse.bass_utils import run_bass_kernel_spmd

    in_maps = [core_inputs(x, wq, wk, c) for c in range(N_CORES)]
    res = run_bass_kernel_spmd(nc, in_maps, core_ids=list(range(N_CORES)), trace=trace)
    _CACHE["last_result"] = res
    gouts = [r["gout"] for r in res.results]
    return combine(gouts)
